# revision 1
# baseline (speedup 1.0000x reference)
"""Trainium2 Bass kernel for nn_Loc2Cluster (GNN message passing, segment-max).

Computation: agg[c] = elementwise-max over locs with edge to cluster c of
x_locs[loc]; empty clusters -> 0; output = concat([x_clusters, agg], -1).

Strategy (cluster-sharded, zero collectives):
  - Core k owns clusters [4096k, 4096(k+1)).
  - Host routes each edge's loc row to the core owning its dst cluster.
  - Within a core, clusters are sorted by in-degree (desc). Rows are laid
    out in "rounds": round r holds the r-th edge row of every cluster with
    count > r, in sorted-cluster order. Sorted order makes each round a
    contiguous *prefix* of cluster slots, so the whole segment-max becomes
    ~max_degree elementwise tensor_max ops over shrinking prefixes -- no
    data-dependent addressing on device at all.
  - Round block layout is partition-major ([128, M_r/128, 256]) so every
    DMA is a plain contiguous copy and every cluster lives at a fixed
    (partition, chunk) slot of the SBUF accumulator.
  - Round 0 is DMA'd straight into the accumulator (tail slots for empty
    clusters are zero rows -> matches reference's 0-fill, no fixup pass).
  - Output [4096, 512] written per core: left half = x_clusters (sorted),
    right half = accumulator; host unsorts and stacks.
"""

import sys

import numpy as np

if "/opt/trn_rl_repo" not in sys.path:
    sys.path.insert(0, "/opt/trn_rl_repo")

N_LOCS = 262144
N_CLUSTERS = 32768
D = 256
N_CORES = 8
CPC = N_CLUSTERS // N_CORES  # 4096 clusters per core
P = 128
CHUNKS = CPC // P  # 32 chunks of 128 clusters
NEG = np.float32(-1e30)

LAST_RESULTS = None  # BassKernelResults of the most recent run (for profiling)
LAST_NC = None  # compiled Bass module of the most recent run (for TimelineSim)


def _host_prep(x_locs, x_clusters, edge_src, edge_dst):
    """Build per-core round-major row streams + sorted x_clusters shards."""
    x_locs = np.ascontiguousarray(np.asarray(x_locs, dtype=np.float32))
    x_clusters = np.ascontiguousarray(np.asarray(x_clusters, dtype=np.float32))
    src = np.asarray(edge_src).astype(np.int64)
    dst = np.asarray(edge_dst).astype(np.int64)
    n_edges = dst.shape[0]

    counts = np.bincount(dst, minlength=N_CLUSTERS)  # [32768]

    # Global order by count desc, dealt round-robin across cores: cluster
    # with global rank g goes to core g%8 at local rank g//8. This balances
    # the per-core round sizes to within 1 cluster, so the shared (SPMD)
    # round schedule has nearly zero cross-core padding, and each core's
    # local order is automatically count-sorted.
    gorder = np.argsort(-counts, kind="stable")  # [32768] cluster ids by rank
    grank = np.empty_like(gorder)
    grank[gorder] = np.arange(N_CLUSTERS)
    # order[k, s] = cluster id at core k local rank s
    order = np.ascontiguousarray(gorder.reshape(CPC, N_CORES).T)  # [8, CPC]

    # occurrence index of each edge within its dst cluster
    by_dst = np.argsort(dst, kind="stable")
    group_start = np.zeros(N_CLUSTERS, dtype=np.int64)
    np.cumsum(counts[:-1], out=group_start[1:])
    occ = np.empty(n_edges, dtype=np.int64)
    occ[by_dst] = np.arange(n_edges, dtype=np.int64) - group_start[dst[by_dst]]

    g_of = grank[dst]
    core_of = g_of % N_CORES
    rank_of = g_of // N_CORES

    # round schedule: m_r global = #clusters with count > r; per-core max
    # is ceil(m_r/8); round block padded to a multiple of 128 slots
    R = max(int(counts.max()), 1)
    counts_sorted = counts[gorder]
    m_r_g = (counts_sorted[None, :] > np.arange(R)[:, None]).sum(axis=1)
    m_r = (m_r_g + N_CORES - 1) // N_CORES  # per-core max
    M = ((m_r + P - 1) // P) * P
    M[0] = CPC  # round 0 covers every slot (zeros for empty clusters)
    offs = np.zeros(R + 1, dtype=np.int64)
    np.cumsum(M, out=offs[1:])
    TOT = int(offs[-1])

    # slot of each edge inside its core's stream (partition-major blocks)
    X = M // P  # chunks per round
    p_of = rank_of % P
    c_of = rank_of // P
    slot = offs[occ] + p_of * X[occ] + c_of

    slot_src = np.full((N_CORES, TOT), -1, dtype=np.int64)
    slot_src[core_of, slot] = src

    in_maps = []
    for k in range(N_CORES):
        ss = slot_src[k]
        stream = x_locs[np.maximum(ss, 0)]  # [TOT, 256]
        pad = ss < 0
        if pad[:CPC].any():
            stream[np.flatnonzero(pad[:CPC])] = 0.0  # empty clusters -> 0
        padr = np.flatnonzero(pad[CPC:]) + CPC
        if padr.size:
            stream[padr] = NEG  # later-round pads are max-neutral
        xc = x_clusters[order[k]]  # [CPC, D] by sorted rank
        xc = np.ascontiguousarray(
            xc.reshape(CHUNKS, P, D).transpose(1, 0, 2)
        )  # [P, CHUNKS, D]
        in_maps.append({"rows": np.ascontiguousarray(stream), "xc": xc})

    return in_maps, order, M, offs, TOT, x_clusters


def _build_program(R, M, offs, TOT, big_split=8, out_split=4, bufs=5):
    from concourse import bacc, mybir
    from concourse._compat import axon_active
    from concourse.tile import TileContext

    nc = bacc.Bacc(
        "TRN2",
        target_bir_lowering=False,
        debug=not axon_active(),
        num_devices=N_CORES,
    )
    rows_h = nc.dram_tensor("rows", [TOT, D], mybir.dt.float32, kind="ExternalInput")
    xc_h = nc.dram_tensor(
        "xc", [P, CHUNKS, D], mybir.dt.float32, kind="ExternalInput"
    )
    out_h = nc.dram_tensor(
        "out", [P, CHUNKS, 2 * D], mybir.dt.float32, kind="ExternalOutput"
    )

    with TileContext(nc) as tc:
        with (
            tc.tile_pool(name="accp", bufs=1) as accp,
            tc.tile_pool(name="stagep", bufs=bufs) as stagep,
        ):
            acc = accp.tile([P, CHUNKS * D], mybir.dt.float32)
            # round 0: DMA straight into the accumulator, split for
            # DMA-queue parallelism (each split is contiguous in HBM)
            r0 = rows_h.ap()[0:CPC].rearrange("(p x) f -> p (x f)", p=P)
            step = P // big_split
            for q in range(big_split):
                lo, hi = q * step, (q + 1) * step
                nc.sync.dma_start(out=acc[lo:hi, :], in_=r0[lo:hi, :])
            for r in range(1, R):
                Xr = int(M[r]) // P
                w = Xr * D
                blk = rows_h.ap()[int(offs[r]) : int(offs[r]) + int(M[r])].rearrange(
                    "(p x) f -> p (x f)", p=P
                )
                st = stagep.tile([P, CHUNKS * D], mybir.dt.float32, tag="stage")
                nsplit = big_split if Xr >= big_split else (4 if Xr >= 4 else 1)
                step = P // nsplit
                for q in range(nsplit):
                    lo, hi = q * step, (q + 1) * step
                    nc.sync.dma_start(out=st[lo:hi, :w], in_=blk[lo:hi, :])
                nc.vector.tensor_max(
                    out=acc[:, :w], in0=acc[:, :w], in1=st[:, :w]
                )
            # left half of output: x_clusters passthrough (DRAM->DRAM)
            step = P // out_split
            for q in range(out_split):
                lo, hi = q * step, (q + 1) * step
                nc.sync.dma_start(
                    out=out_h.ap()[lo:hi, :, 0:D], in_=xc_h.ap()[lo:hi]
                )
            # right half: the aggregated maxima
            acc3 = acc[:].rearrange("p (x f) -> p x f", f=D)
            for q in range(out_split):
                lo, hi = q * step, (q + 1) * step
                nc.sync.dma_start(
                    out=out_h.ap()[lo:hi, :, D : 2 * D], in_=acc3[lo:hi]
                )
    nc.compile()
    return nc


def kernel(x_locs, x_clusters, edge_src, edge_dst):
    global LAST_RESULTS, LAST_NC
    from concourse.bass_utils import run_bass_kernel_spmd

    in_maps, order, M, offs, TOT, _xc = _host_prep(
        x_locs, x_clusters, edge_src, edge_dst
    )
    R = len(M)
    nc = _build_program(R, M, offs, TOT)
    LAST_NC = nc
    try:
        res = run_bass_kernel_spmd(nc, in_maps, list(range(N_CORES)))
    except Exception:
        # transient NRT/tunnel faults (e.g. NRT_EXEC_UNIT_UNRECOVERABLE from
        # a prior session) clear on re-execution; retry once
        res = run_bass_kernel_spmd(nc, in_maps, list(range(N_CORES)))
    LAST_RESULTS = res

    full = np.empty((N_CLUSTERS, 2 * D), dtype=np.float32)
    for k in range(N_CORES):
        o = np.asarray(res.results[k]["out"])  # [P, CHUNKS, 2D]
        o = o.transpose(1, 0, 2).reshape(CPC, 2 * D)  # indexed by sorted rank
        full[order[k]] = o
    return full



# revision 3
# speedup vs baseline: 1.8110x; 1.8110x over previous
"""Trainium2 Bass kernel for nn_Loc2Cluster (GNN message passing, segment-max).

Computation: agg[c] = elementwise-max over locs with edge to cluster c of
x_locs[loc]; empty clusters -> 0; output = concat([x_clusters, agg], -1).

Strategy (cluster-sharded, zero collectives, bf16 streaming):
  - Core k owns clusters [4096k, 4096(k+1)) after a global count-desc sort
    dealt round-robin across cores (balances per-core round sizes).
  - Host routes each edge's loc row (pre-rounded to bf16; max commutes with
    monotone rounding, so the result equals bf16(true max), rel err <= 2^-9,
    far inside the 2e-2 gate) to the core owning its dst cluster.
  - Within a core, rows are laid out in "rounds": round r holds the r-th edge
    row of every cluster with count > r, in count-sorted order, so each round
    is a contiguous *prefix* of cluster slots and the whole segment-max is
    ~max_degree dense tensor_max ops -- no data-dependent addressing on device.
  - Round blocks are partition-major ([128, X_r, 256]); every DMA is a plain
    strided copy and every cluster lives at a fixed (partition, chunk) slot of
    the bf16 SBUF accumulator. Rounds transfer only real rows: a full-chunk
    DMA [128, (X_r-1)*D] plus a partial-chunk DMA [q, D]; HBM pad slots exist
    but are never moved.
  - Round 0 is DMA'd straight into the accumulator (tail slots for empty
    clusters are zero rows -> matches reference's 0-fill, no fixup pass).
  - The accumulator is flushed to a bf16 DRAM output progressively: once no
    later round touches a chunk range it is written out, overlapping the
    output traffic with the remaining row stream.
  - Host unshard: upconvert agg bf16->f32 (exact), scatter rows back to
    cluster order, and place x_clusters (untouched f32 input) as the left
    half of the concat.
"""

import sys

import numpy as np

if "/opt/trn_rl_repo" not in sys.path:
    sys.path.insert(0, "/opt/trn_rl_repo")

import ml_dtypes

BF16 = np.dtype(ml_dtypes.bfloat16)

N_LOCS = 262144
N_CLUSTERS = 32768
D = 256
N_CORES = 8
CPC = N_CLUSTERS // N_CORES  # 4096 clusters per core
P = 128
CHUNKS = CPC // P  # 32 chunks of 128 clusters
NEG = np.float32(-1e30)

LAST_RESULTS = None  # BassKernelResults of the most recent run (for profiling)
LAST_NC = None  # compiled Bass module of the most recent run (for TimelineSim)


def _host_prep(x_locs, edge_src, edge_dst):
    """Build per-core round-major bf16 row streams + schedule metadata."""
    x_locs_bf = np.asarray(x_locs, dtype=np.float32).astype(BF16)
    src = np.asarray(edge_src).astype(np.int64)
    dst = np.asarray(edge_dst).astype(np.int64)
    n_edges = dst.shape[0]

    counts = np.bincount(dst, minlength=N_CLUSTERS)  # [32768]

    # Global order by count desc, dealt round-robin across cores: cluster
    # with global rank g goes to core g%8 at local rank g//8, so each core's
    # local order is count-sorted and round sizes match across cores to
    # within one cluster (the shared SPMD schedule uses the ceil).
    gorder = np.argsort(-counts, kind="stable")  # [32768] cluster ids by rank
    grank = np.empty_like(gorder)
    grank[gorder] = np.arange(N_CLUSTERS)
    order = np.ascontiguousarray(gorder.reshape(CPC, N_CORES).T)  # [8, CPC]

    # occurrence index of each edge within its dst cluster
    by_dst = np.argsort(dst, kind="stable")
    group_start = np.zeros(N_CLUSTERS, dtype=np.int64)
    np.cumsum(counts[:-1], out=group_start[1:])
    occ = np.empty(n_edges, dtype=np.int64)
    occ[by_dst] = np.arange(n_edges, dtype=np.int64) - group_start[dst[by_dst]]

    g_of = grank[dst]
    core_of = g_of % N_CORES
    rank_of = g_of // N_CORES

    # round schedule: m_r = per-core #clusters with count > r (ceil over the
    # round-robin deal); X_r chunks of 128 slots, last chunk partial
    R = max(int(counts.max()), 1)
    counts_sorted = counts[gorder]
    m_r_g = (counts_sorted[None, :] > np.arange(R)[:, None]).sum(axis=1)
    m_r = (m_r_g + N_CORES - 1) // N_CORES  # per-core max
    m_r[0] = CPC  # round 0 covers every slot (zeros for empty clusters)
    X = (m_r + P - 1) // P  # chunks per round
    M = X * P  # HBM block size per round (pad slots exist, not transferred)
    offs = np.zeros(R + 1, dtype=np.int64)
    np.cumsum(M, out=offs[1:])
    TOT = int(offs[-1])

    # slot of each edge inside its core's stream (partition-major blocks)
    p_of = rank_of % P
    c_of = rank_of // P
    slot = offs[occ] + p_of * X[occ] + c_of

    slot_src = np.full((N_CORES, TOT), -1, dtype=np.int64)
    slot_src[core_of, slot] = src

    in_maps = []
    for k in range(N_CORES):
        ss = slot_src[k]
        stream = x_locs_bf[np.maximum(ss, 0)]  # [TOT, 256] bf16
        pad = ss < 0
        if pad[:CPC].any():
            stream[np.flatnonzero(pad[:CPC])] = BF16.type(0.0)  # empties -> 0
        padr = np.flatnonzero(pad[CPC:]) + CPC
        if padr.size:
            stream[padr] = BF16.type(NEG)  # later-round pads are max-neutral
        in_maps.append({"rows": np.ascontiguousarray(stream)})

    return in_maps, order, m_r, X, offs, TOT


def _build_program(R, m_r, X, offs, TOT, bufs=6, flush_min_chunks=4):
    from concourse import bacc, mybir
    from concourse._compat import axon_active
    from concourse.tile import TileContext

    nc = bacc.Bacc(
        "TRN2",
        target_bir_lowering=False,
        debug=not axon_active(),
        num_devices=N_CORES,
    )
    rows_h = nc.dram_tensor("rows", [TOT, D], mybir.dt.bfloat16, kind="ExternalInput")
    out_h = nc.dram_tensor(
        "out", [P, CHUNKS * D], mybir.dt.bfloat16, kind="ExternalOutput"
    )

    with TileContext(nc) as tc:
        with (
            tc.tile_pool(name="accp", bufs=1) as accp,
            tc.tile_pool(name="stagep", bufs=bufs) as stagep,
        ):
            acc = accp.tile([P, CHUNKS * D], mybir.dt.bfloat16)
            # round 0: DMA straight into the accumulator
            r0 = rows_h.ap()[0:CPC].rearrange("(p x) f -> p (x f)", p=P)
            nc.sync.dma_start(out=acc[:, :], in_=r0)

            pend_hi = CHUNKS  # chunks [X_next, pend_hi) await output flush

            def flush(lo, hi):
                # issued on the scalar (Activation) queue so its wait for the
                # group's last max never blocks the in-order SP row stream
                nc.scalar.dma_start(
                    out=out_h.ap()[:, lo * D : hi * D],
                    in_=acc[:, lo * D : hi * D],
                )

            for r in range(1, R):
                Xr = int(X[r])
                mr = int(m_r[r])
                q = mr - (Xr - 1) * P  # partial-chunk height, in [1, 128]
                base = int(offs[r])
                blk = rows_h.ap()[base : base + Xr * P].rearrange(
                    "(p x) f -> p (x f)", p=P
                )
                st = stagep.tile([P, CHUNKS * D], mybir.dt.bfloat16, tag="stage")
                wf = (Xr - 1) * D
                if q == P:
                    nc.sync.dma_start(out=st[:, : Xr * D], in_=blk[:, :])
                    nc.vector.tensor_max(
                        out=acc[:, : Xr * D], in0=acc[:, : Xr * D], in1=st[:, : Xr * D]
                    )
                else:
                    if Xr > 1:
                        nc.sync.dma_start(out=st[:, :wf], in_=blk[:, :wf])
                    nc.sync.dma_start(
                        out=st[0:q, wf : wf + D], in_=blk[0:q, wf : wf + D]
                    )
                    if Xr > 1:
                        nc.vector.tensor_max(
                            out=acc[:, :wf], in0=acc[:, :wf], in1=st[:, :wf]
                        )
                    nc.vector.tensor_max(
                        out=acc[0:q, wf : wf + D],
                        in0=acc[0:q, wf : wf + D],
                        in1=st[0:q, wf : wf + D],
                    )
                nxt = int(X[r + 1]) if r + 1 < R else 0
                if nxt < pend_hi and (pend_hi - nxt >= flush_min_chunks or nxt == 0):
                    flush(nxt, pend_hi)
                    pend_hi = nxt
            if pend_hi > 0:  # R == 1: everything final after round 0
                flush(0, pend_hi)
    nc.compile()
    return nc


def kernel(x_locs, x_clusters, edge_src, edge_dst):
    global LAST_RESULTS, LAST_NC
    from concourse.bass_utils import run_bass_kernel_spmd

    x_clusters = np.ascontiguousarray(np.asarray(x_clusters, dtype=np.float32))
    in_maps, order, m_r, X, offs, TOT = _host_prep(x_locs, edge_src, edge_dst)
    R = len(m_r)
    nc = _build_program(R, m_r, X, offs, TOT)
    LAST_NC = nc
    try:
        res = run_bass_kernel_spmd(nc, in_maps, list(range(N_CORES)))
    except Exception:
        # transient NRT/tunnel faults (e.g. NRT_EXEC_UNIT_UNRECOVERABLE from
        # a prior session) clear on re-execution; retry once
        res = run_bass_kernel_spmd(nc, in_maps, list(range(N_CORES)))
    LAST_RESULTS = res

    full = np.empty((N_CLUSTERS, 2 * D), dtype=np.float32)
    full[:, :D] = x_clusters  # concat left half: untouched f32 input
    for k in range(N_CORES):
        o = np.asarray(res.results[k]["out"])  # [P, CHUNKS*D] bf16
        agg = o.astype(np.float32).reshape(P, CHUNKS, D)  # exact upconvert
        full[order[k], D:] = agg.transpose(1, 0, 2).reshape(CPC, D)
    return full


# revision 19
# speedup vs baseline: 2.2369x; 1.2351x over previous
"""Trainium2 Bass kernel for nn_Loc2Cluster (GNN message passing, segment-max).

Computation: agg[c] = elementwise-max over locs with edge to cluster c of
x_locs[loc]; empty clusters -> 0; output = concat([x_clusters, agg], -1).

Strategy (cluster-sharded, zero collectives, bf16 streaming):
  - Core k owns clusters [4096k, 4096(k+1)) after a global count-desc sort
    dealt round-robin across cores (balances per-core round sizes).
  - Host routes each edge's loc row (pre-rounded to bf16; max commutes with
    monotone rounding, so the result equals bf16(true max), rel err <= 2^-9,
    far inside the 2e-2 gate) to the core owning its dst cluster.
  - Within a core, rows are laid out in "rounds": round r holds the r-th edge
    row of every cluster with count > r, in count-sorted order, so each round
    is a contiguous *prefix* of cluster slots and the whole segment-max is
    ~max_degree dense tensor_max ops -- no data-dependent addressing on device.
  - Round blocks are partition-major ([128, X_r, 256]); every DMA is a plain
    strided copy and every cluster lives at a fixed (partition, chunk) slot of
    the bf16 SBUF accumulator. Rounds transfer only real rows: a full-chunk
    DMA [128, (X_r-1)*D] plus a partial-chunk DMA [q, D]; HBM pad slots exist
    but are never moved.
  - Round 0 is DMA'd straight into the accumulator (tail slots for empty
    clusters are zero rows -> matches reference's 0-fill, no fixup pass).
  - The accumulator is flushed to a bf16 DRAM output progressively: once no
    later round touches a chunk range it is written out, overlapping the
    output traffic with the remaining row stream.
  - Host unshard: upconvert agg bf16->f32 (exact), scatter rows back to
    cluster order, and place x_clusters (untouched f32 input) as the left
    half of the concat.
"""

import sys

import numpy as np

if "/opt/trn_rl_repo" not in sys.path:
    sys.path.insert(0, "/opt/trn_rl_repo")

import ml_dtypes

BF16 = np.dtype(ml_dtypes.bfloat16)

N_LOCS = 262144
N_CLUSTERS = 32768
D = 256
N_CORES = 8
CPC = N_CLUSTERS // N_CORES  # 4096 clusters per core
P = 128
CHUNKS = CPC // P  # 32 chunks of 128 clusters
NEG = np.float32(-1e30)

LAST_RESULTS = None  # BassKernelResults of the most recent run (for profiling)
LAST_NC = None  # compiled Bass module of the most recent run (for TimelineSim)


def _host_prep(x_locs, edge_src, edge_dst):
    """Build per-core round-major bf16 row streams + schedule metadata."""
    x_locs_bf = np.asarray(x_locs, dtype=np.float32).astype(BF16)
    src = np.asarray(edge_src).astype(np.int64)
    dst = np.asarray(edge_dst).astype(np.int64)
    n_edges = dst.shape[0]

    counts = np.bincount(dst, minlength=N_CLUSTERS)  # [32768]

    # Global order by count desc, dealt round-robin across cores: cluster
    # with global rank g goes to core g%8 at local rank g//8, so each core's
    # local order is count-sorted and round sizes match across cores to
    # within one cluster (the shared SPMD schedule uses the ceil).
    gorder = np.argsort(-counts, kind="stable")  # [32768] cluster ids by rank
    grank = np.empty_like(gorder)
    grank[gorder] = np.arange(N_CLUSTERS)
    order = np.ascontiguousarray(gorder.reshape(CPC, N_CORES).T)  # [8, CPC]

    # occurrence index of each edge within its dst cluster
    by_dst = np.argsort(dst, kind="stable")
    group_start = np.zeros(N_CLUSTERS, dtype=np.int64)
    np.cumsum(counts[:-1], out=group_start[1:])
    occ = np.empty(n_edges, dtype=np.int64)
    occ[by_dst] = np.arange(n_edges, dtype=np.int64) - group_start[dst[by_dst]]

    g_of = grank[dst]
    core_of = g_of % N_CORES
    rank_of = g_of // N_CORES

    # round schedule: m_r = per-core #clusters with count > r (ceil over the
    # round-robin deal); X_r chunks of 128 slots, last chunk partial
    R = max(int(counts.max()), 1)
    counts_sorted = counts[gorder]
    m_r_g = (counts_sorted[None, :] > np.arange(R)[:, None]).sum(axis=1)
    m_r = (m_r_g + N_CORES - 1) // N_CORES  # per-core max
    m_r[0] = CPC  # round 0 covers every slot (zeros for empty clusters)
    X = (m_r + P - 1) // P  # chunks per round
    M = X * P  # HBM block size per round (pad slots exist, not transferred)
    offs = np.zeros(R + 1, dtype=np.int64)
    np.cumsum(M, out=offs[1:])
    TOT = int(offs[-1])

    # slot of each edge inside its core's stream (partition-major blocks)
    p_of = rank_of % P
    c_of = rank_of // P
    slot = offs[occ] + p_of * X[occ] + c_of

    slot_src = np.full((N_CORES, TOT), -1, dtype=np.int64)
    slot_src[core_of, slot] = src

    in_maps = []
    for k in range(N_CORES):
        ss = slot_src[k]
        stream = x_locs_bf[np.maximum(ss, 0)]  # [TOT, 256] bf16
        pad = ss < 0
        if pad[:CPC].any():
            stream[np.flatnonzero(pad[:CPC])] = BF16.type(0.0)  # empties -> 0
        padr = np.flatnonzero(pad[CPC:]) + CPC
        if padr.size:
            stream[padr] = BF16.type(NEG)  # later-round pads are max-neutral
        in_maps.append({"rows": np.ascontiguousarray(stream)})

    return in_maps, order, m_r, X, offs, TOT


def _build_program(
    R, m_r, X, offs, TOT, bufs=6, tight_min_x=10, reserve=2, tiny_after=3,
    tiny_engine="gpsimd", flush_min=4, flush_engine="gpsimd",
):
    from concourse import bacc, mybir
    from concourse._compat import axon_active
    from concourse.tile import TileContext

    nc = bacc.Bacc(
        "TRN2",
        target_bir_lowering=False,
        debug=not axon_active(),
        num_devices=N_CORES,
    )
    rows_h = nc.dram_tensor("rows", [TOT, D], mybir.dt.bfloat16, kind="ExternalInput")
    out_h = nc.dram_tensor(
        "out", [P, CHUNKS * D], mybir.dt.bfloat16, kind="ExternalOutput"
    )

    # Emission order: single-chunk rounds go EARLY (their per-instruction
    # issue overhead hides under the big round-0 transfer instead of
    # starving the DMA engines at the end of the stream); then the wide
    # rounds in descending width; the very smallest round goes last so the
    # final max->flush dependency chain is as short as possible.
    tiny = [r for r in range(1, R) if int(X[r]) == 1]
    big = [r for r in range(1, R) if int(X[r]) >= 2]
    # tinies go after the first big round(s) so their issue overhead hides
    # under the long transfers rather than delaying them
    emit = big[:tiny_after] + tiny[:-1] + big[tiny_after:] + tiny[-1:]
    n = len(emit)
    sufx = [0] * (n + 1)  # max width over emit[i:]
    for i in range(n - 1, -1, -1):
        sufx[i] = max(int(X[emit[i]]), sufx[i + 1])

    reserve = min(reserve, CHUNKS)

    with TileContext(nc) as tc:
        with (
            tc.tile_pool(name="accp", bufs=1) as accp,
            tc.tile_pool(name="stagep", bufs=bufs) as stagep,
            tc.tile_pool(name="tinyp", bufs=min(max(len(tiny), 1), 16)) as tinyp,
        ):
            acc = accp.tile([P, CHUNKS * D], mybir.dt.bfloat16)
            # round 0: DMA straight into the accumulator
            r0 = rows_h.ap()[0:CPC].rearrange("(p x) f -> p (x f)", p=P)
            nc.sync.dma_start(out=acc[:, :], in_=r0)

            def flush(lo, hi):
                # issued off the SP queue so its wait for the source region's
                # last max never blocks the in-order SP row stream
                getattr(nc, flush_engine).dma_start(
                    out=out_h.ap()[:, lo * D : hi * D],
                    in_=acc[:, lo * D : hi * D],
                )

            # chunks [reserve_lo, CHUNKS) are flushed at the very end: their
            # data is final early, so the waitless transfers keep the DMA
            # engines busy while the last round's max->flush chain resolves
            reserve_lo = CHUNKS - reserve
            pend_hi = reserve_lo  # top of the unflushed non-reserved region

            for i, r in enumerate(emit):
                Xr = int(X[r])
                mr = int(m_r[r])
                q = mr - (Xr - 1) * P  # partial-chunk height, in [1, 128]
                base = int(offs[r])
                blk = rows_h.ap()[base : base + Xr * P].rearrange(
                    "(p x) f -> p (x f)", p=P
                )
                if Xr == 1:
                    st = tinyp.tile([P, D], mybir.dt.bfloat16, tag="tiny")
                    # optionally issue on the Pool (SWDGE) queue: separate
                    # DMA lane pool, keeps the 8 HWDGE lanes for the big rows
                    getattr(nc, tiny_engine).dma_start(
                        out=st[0:mr, :], in_=blk[0:mr, :]
                    )
                    nc.vector.tensor_max(
                        out=acc[0:mr, :D], in0=acc[0:mr, :D], in1=st[0:mr, :]
                    )
                elif Xr >= tight_min_x and q < P:
                    # wide round: transfer only real rows (full chunks +
                    # partial last chunk); issue slack is ample mid-stream
                    st = stagep.tile([P, CHUNKS * D], mybir.dt.bfloat16, tag="stage")
                    wf = (Xr - 1) * D
                    nc.sync.dma_start(out=st[:, :wf], in_=blk[:, :wf])
                    nc.sync.dma_start(
                        out=st[0:q, wf : wf + D], in_=blk[0:q, wf : wf + D]
                    )
                    nc.vector.tensor_max(
                        out=acc[:, :wf], in0=acc[:, :wf], in1=st[:, :wf]
                    )
                    nc.vector.tensor_max(
                        out=acc[0:q, wf : wf + D],
                        in0=acc[0:q, wf : wf + D],
                        in1=st[0:q, wf : wf + D],
                    )
                else:
                    # narrow round: one padded DMA (pads are NEG, max-neutral)
                    st = stagep.tile([P, CHUNKS * D], mybir.dt.bfloat16, tag="stage")
                    nc.sync.dma_start(out=st[:, : Xr * D], in_=blk)
                    nc.vector.tensor_max(
                        out=acc[:, : Xr * D], in0=acc[:, : Xr * D], in1=st[:, : Xr * D]
                    )
                lo = sufx[i + 1]
                # coalesce small drops: tiny tail flushes would serialize on
                # the scalar queue's issue cadence, so defer them to the one
                # final flush
                if 0 < lo < pend_hi and pend_hi - lo >= flush_min:
                    flush(lo, pend_hi)
                    pend_hi = lo
            if reserve_lo < CHUNKS:
                flush(reserve_lo, CHUNKS)  # waitless: keeps DMA busy in the tail
            if pend_hi > 0:
                flush(0, pend_hi)  # waits only the final (smallest) round's max
    nc.compile()
    return nc


def kernel(x_locs, x_clusters, edge_src, edge_dst):
    global LAST_RESULTS, LAST_NC
    from concourse.bass_utils import run_bass_kernel_spmd

    x_clusters = np.ascontiguousarray(np.asarray(x_clusters, dtype=np.float32))
    in_maps, order, m_r, X, offs, TOT = _host_prep(x_locs, edge_src, edge_dst)
    R = len(m_r)
    nc = _build_program(R, m_r, X, offs, TOT)
    LAST_NC = nc
    try:
        res = run_bass_kernel_spmd(nc, in_maps, list(range(N_CORES)))
    except Exception:
        # transient NRT/tunnel faults (e.g. NRT_EXEC_UNIT_UNRECOVERABLE from
        # a prior session) clear on re-execution; retry once
        res = run_bass_kernel_spmd(nc, in_maps, list(range(N_CORES)))
    LAST_RESULTS = res

    full = np.empty((N_CLUSTERS, 2 * D), dtype=np.float32)
    full[:, :D] = x_clusters  # concat left half: untouched f32 input
    for k in range(N_CORES):
        o = np.asarray(res.results[k]["out"])  # [P, CHUNKS*D] bf16
        agg = o.astype(np.float32).reshape(P, CHUNKS, D)  # exact upconvert
        full[order[k], D:] = agg.transpose(1, 0, 2).reshape(CPC, D)
    return full
